# revision 2
# baseline (speedup 1.0000x reference)
"""Annular patch embedding on 8 TRN2 NeuronCores.

Math: tokens[b, r, d] = sum_p x[b, p] * mask[r, p] * W[d, p]; out = tokens @
fc_w.T + fc_b. The rings are disjoint, so this is a segmented matmul over only
the ~39.4K pixels covered by rings. The fc projection is folded into the conv
weights on the host: V[o, p] = sum_d fc_w[o, d] * W[d, p], so the device
computes out[b, r, o] = sum_{p in ring r} x[b, p] * V[o, p] (+ bias via a
synthetic pixel with x == 1 and V column == fc_b).

Distribution: ring-sorted pixels are packed into 128-pixel contraction tiles,
40 tiles per core (8 cores x 40 = 320 slots for the 316 real tiles). Each core
runs the same SPMD graph: 5 PSUM accumulation groups with fixed tile counts
(19, 9, 6, 4, 2); a ring occupies an exact set of (core, group) slots, and the
host sums the per-slot partial outputs. The packing below covers every ring's
tile count exactly, so there is no zero-padding waste beyond the partial last
tile of each ring. No collectives are needed: every input byte is read by
exactly one core and the cross-piece reduction is a cheap host-side add.

The device graph is hand-scheduled raw Bass (no TileContext). Per core: input
chunks (x and V columns fused in consumption order) stream over BOTH HWDGE
rings (Sync + Scalar), alternating chunks at an even cadence — a [128, C]
transfer is 128 line-packets round-robined over the core's 16 DMA engines,
and per-ring throughput is line-dispatch-limited, so the rings must carry
disjoint column ranges (partition-split halves measured ~35% slower per
engine-packet) and no single early chunk may sit ahead of urgent data on the
late-waking Scalar ring. fp16 matmuls chase the chunk stream into per-group
PSUM banks; a group's completion semaphore is raised by the NEXT group's
first matmul (or a PE drain for the final group) so psum readers never race
the PE's drain. The psum->f16 copies run split across the vector and scalar
engines in parallel (a copy costs a ~flat ~350 ns regardless of partition
count; scalar's first ACTIVATE pays a one-time ~1.3 us ACT_TABLE_LOAD, so a
dummy copy prewarms it mid-stream), into a [32, 1920] staging layout that
halves the output DMA line count; one 16-line output DMA per ring ships the
result. After the end-of-block barrier, gpsimd alone waits for the output
DMA receipts (so the runtime cannot read back stale output) and clears the
semaphores so the NEFF stays re-executable.

Clock management (HAM): the core defaults to half clock; ~3.4 us of
continuous engine busy triggers a boost cycle (3.4 us full clock, then a
forced ~6.8 us half-clock cooldown, ~1/3 duty). Long PE warm-up blocks enter
that cycle early and land the cooldown on the stream tail — measured net
negative, so this kernel keeps warm-ups minimal and stays out of the cycle.
"""

import numpy as np
import ml_dtypes

import concourse.bass as bass
import concourse.mybir as mybir
import concourse.tile as tile
from concourse import bacc
from concourse.bass_utils import run_bass_kernel_spmd

IMG = 224
NPIX = IMG * IMG
B = 64
TOKEN_DIM = 256
OUT_DIM = 192
NUM_RINGS = 16
N_CORES = 8
P = 128

# PSUM accumulation groups per core (tiles per group); identical on all cores.
# Ordered small-to-big: group g's psum-copy is released by group g+1's FIRST
# matmul (the drain releases only the last group), so with the big group
# last, every other group's copies fire tiles earlier — with big-to-small,
# the second-to-last group's signal lands ~2 tiles before the drain and its
# copies serialize with the final group's on the tail-critical copy engines.
# The PE work remaining after the last input chunk is a chunk property, not
# a group property, so this reorder costs nothing there.
GROUP_SIZES = (2, 4, 6, 9, 19)
T_CORE = sum(GROUP_SIZES)  # 40 tiles of 128 pixels per core
N_GROUPS = len(GROUP_SIZES)

# Ring r (tile counts 2,4,6,9,11,14,16,19,21,23,26,28,31,33,35,38) is split
# into pieces whose sizes are drawn from the per-core group sizes. Each piece
# occupies one (core, group) slot. Slot budget: 8 of each size; this table
# uses 6/8/8/8/8 of sizes 2/4/6/9/19 — an exact cover.
RING_DECOMP = (
    (2,), (4,), (6,), (9,),
    (2, 9), (2, 4, 4, 4), (2, 4, 4, 6), (19,),
    (2, 19), (4, 19), (2, 6, 9, 9), (9, 19),
    (6, 6, 19), (6, 9, 9, 9), (4, 6, 6, 19), (19, 19),
)

COMPUTE_DTYPE = "f16"  # "f16", "bf16", or "f32": f16 is the same
# speed as bf16 (2 bytes, full-rate PE) but has 10 mantissa bits, cutting the
# quantization error ~8x. All values here are far inside f16 range.
MODE = "raw"  # "raw" (hand-scheduled Block) or "tile" (TileContext)
# Input tiles (x columns + V columns interleaved per chunk) are DMA'd in
# these chunks, pipelined against the matmul stream over the two HWDGE rings
# (Sync = "a", Scalar = "b"). A [128, C] transfer becomes 128 line-packets of
# C*2 bytes fanned over the core's 16 DMA engines; per-ring throughput is
# line-dispatch-limited (~11-15 ns/line at full clock, ~2x that in HAM
# half-clock windows) until the combined ~400-430 GB/s HBM cap. The PE
# consumes ~one tile per 120-160 ns — nearly the same pace the wire delivers
# one (149 ns at 430 GB/s) — so the chunks must alternate rings at an even
# cadence: a big early chunk on the late-waking Scalar ring stalls the PE in
# a way it never recovers from (measured +5 us). Lines >= 3 KB keep both
# rings near the cap through throttle windows; the last chunk is small so
# little PE work remains after the final byte lands. Mode "s" (partition-
# split across rings) is supported but measured slower per engine-packet —
# unused.
CHUNK_PLAN = ((2, "a"), (4, "b"), (6, "a"), (8, "b"), (8, "a"), (6, "b"), (4, "a"), (2, "b"))
CHUNK_TILES = tuple(t for t, _ in CHUNK_PLAN)
WARMUP_MMS = 12  # a few matmuls to fill the PE pipeline before real work.
# NOTE: long warmup blocks (26+) were load-bearing in an earlier version —
# ~3.4 us of continuous busy triggers the HAM boost cycle (3.4 us full clock,
# then ~6.8 us forced half clock). Measured end to end, entering that cycle
# is net NEGATIVE here: the cooldown lands on the stream tail. Attempts to
# phase the boost onto the tail with gpsimd busy-work (HAM_COLS > 0) were
# within noise, so the kernel now stays out of the boost cycle entirely.
HAM_COLS = 0  # busy_sb columns for the gpsimd HAM-phasing memset (disabled)
OUT_DT = "f16"  # output staging dtype: "f16" halves the out DMA, err ~5e-4
TILE_COLS = B + OUT_DIM  # 256 fused columns per tile (64 x + 192 V)
# Output staging packs [B=64, 5*192] as [32, 2*5*192]: batch row b lives on
# partition b%32, group g in the column block (2*g + b//32)*192. Halving the
# partition count halves the output DMA line count (16 lines per ring after
# the row split), and interleaving the halves keeps groups 0-3 contiguous in
# columns so they can ship while group 4 finishes. A psum->f16 copy costs a
# ~flat ~350 ns regardless of partition count, so the vector engine copies
# half b//32==0 and the scalar engine copies b//32==1 in parallel.
OUT_P = 32
OUT_COLS = 2 * N_GROUPS * OUT_DIM

# test.py hooks: extra kwargs for run_bass_kernel_spmd (e.g. trace=True), and
# the last BassKernelResults for timing introspection.
_RUN_KWARGS = {}
LAST_RESULTS = None

_GRAPH_CACHE = {}


def _chunk_bounds():
    """(t0, t1) tile ranges per DMA chunk."""
    assert sum(CHUNK_TILES) == T_CORE
    bounds, t = [], 0
    for ch in CHUNK_TILES:
        bounds.append((t, t + ch))
        t += ch
    return bounds


def _sb_offsets():
    """Per-tile column offsets of the x block and V block in the fused
    [128, T_CORE * TILE_COLS] layout: chunk c holds its tiles' x columns
    first, then its tiles' V columns, so DMA arrival order == use order."""
    xoff, voff = [0] * T_CORE, [0] * T_CORE
    for t0, t1 in _chunk_bounds():
        base = t0 * TILE_COLS
        for t in range(t0, t1):
            xoff[t] = base + (t - t0) * B
            voff[t] = base + (t1 - t0) * B + (t - t0) * OUT_DIM
    return xoff, voff


def _build_graph_raw(dt):
    # NOTE: skipping the constructor-emitted all-engine barrier (to start
    # the entry DMA triggers earlier) was tried and measured ~2 us WORSE —
    # without it the Sync queue's first packets actually land later.
    out_dt = mybir.dt.float16 if OUT_DT == "f16" else mybir.dt.float32
    nc = bass.Bass("TRN2", debug=False, num_devices=N_CORES)
    data = nc.declare_dram_parameter(
        "data", [P, T_CORE * TILE_COLS], dt, isOutput=False
    )
    out = nc.declare_dram_parameter("out", [OUT_P, OUT_COLS], out_dt, isOutput=True)

    data_sb = nc.alloc_sbuf_tensor("data_sb", [P, T_CORE * TILE_COLS], dt)
    out_sb = nc.alloc_sbuf_tensor("out_sb", [OUT_P, OUT_COLS], out_dt)
    warm_sb = nc.alloc_sbuf_tensor("warm_sb", [P, B + 128], dt)
    busy_sb = (
        nc.alloc_sbuf_tensor("busy_sb", [P, HAM_COLS], dt) if HAM_COLS else None
    )

    pss = [
        nc.alloc_psum_tensor(f"ps{g}", [B, OUT_DIM], mybir.dt.float32)
        for g in range(N_GROUPS)
    ]
    warm_ps = nc.alloc_psum_tensor("warm_ps", [B, 128], mybir.dt.float32)

    a_sem = nc.alloc_semaphore("a_sem")  # Sync-ring input receipts
    b_sem = nc.alloc_semaphore("b_sem")  # Scalar-ring input receipts
    mm_sem = nc.alloc_semaphore("mm_sem")
    copy_a_sem = nc.alloc_semaphore("copy_a_sem")  # vector (half 0) copies
    copy_b_sem = nc.alloc_semaphore("copy_b_sem")  # scalar (half 1) copies
    # Completion sem for the four output DMAs: only gpsimd waits on it, after
    # the end-of-block barrier, so NEFF completion implies the output landed.
    out_sem = nc.alloc_semaphore("out_sem")
    sem_nums = sorted(
        s.num for s in (a_sem, b_sem, mm_sem, copy_a_sem, copy_b_sem, out_sem)
    )
    assert sem_nums == list(range(sem_nums[0], sem_nums[0] + 6))
    sem_range = range(sem_nums[0], sem_nums[-1] + 1)

    chunks = _chunk_bounds()
    xoff, voff = _sb_offsets()
    g4_col = 2 * (N_GROUPS - 1) * OUT_DIM  # groups 0-3 (both halves) before it
    HALF = P // 2

    # Per-chunk transfer bookkeeping: which rings carry it, partition range
    # per ring, and the cumulative per-ring completion thresholds the PE must
    # wait for (transfers complete FIFO per ring).
    a_cnt = b_cnt = 0
    chunk_xfers = []  # chunk -> list of (ring, lo, hi)
    chunk_waits = []  # chunk -> list of (sem, threshold)
    for tiles, mode in CHUNK_PLAN:
        xf, waits = [], []
        if mode in ("a", "s"):
            a_cnt += 1
            xf.append(("a", 0, HALF if mode == "s" else P))
            waits.append((a_sem, 16 * a_cnt))
        if mode in ("b", "s"):
            b_cnt += 1
            xf.append(("b", HALF if mode == "s" else 0, P))
            waits.append((b_sem, 16 * b_cnt))
        chunk_xfers.append(xf)
        chunk_waits.append(waits)

    def _chunk_dma(eng, c, ring):
        t0, t1 = chunks[c]
        for r, lo, hi in chunk_xfers[c]:
            if r == ring:
                eng.dma_start(
                    data_sb[lo:hi, t0 * TILE_COLS : t1 * TILE_COLS],
                    data[lo:hi, t0 * TILE_COLS : t1 * TILE_COLS],
                ).then_inc(a_sem if ring == "a" else b_sem, 16)

    def _copy(eng, g, half, sem):
        dst = out_sb[:, (2 * g + half) * OUT_DIM : (2 * g + half + 1) * OUT_DIM]
        src = pss[g][half * OUT_P : (half + 1) * OUT_P, :]
        op = eng.tensor_copy(dst, src) if hasattr(eng, "tensor_copy") else eng.copy(dst, src)
        op.then_inc(sem, 1)

    def _ring_chunks(ring):
        return [c for c in range(len(chunks)) if any(r == ring for r, _, _ in chunk_xfers[c])]

    # Issue the first chunk of each ring from the entry basic block, ahead
    # of the Block-entry branch, so the DMA pipeline starts as early as
    # possible. (A tiny early "wake" transfer for the slow-starting Scalar
    # ring was tried and did not move its ~1.4 us first-packet lag.)
    _chunk_dma(nc.sync, _ring_chunks("a")[0], "a")
    _chunk_dma(nc.scalar, _ring_chunks("b")[0], "b")

    with nc.Block(no_gpsimd_drain=True) as block:

        @block.sync
        def _(sync):
            for c in _ring_chunks("a")[1:]:
                _chunk_dma(sync, c, "a")
            # One output DMA per ring (16 lines x 3840 B). Splitting out an
            # "early" groups-0-3 DMA to overlap the PE drain was tried three
            # ways (v5, v8) and always measured worse: the second ~620 ns
            # trigger on the tail queues costs more than the overlap saves.
            sync.wait_ge(copy_a_sem, N_GROUPS)
            sync.wait_ge(copy_b_sem, N_GROUPS)
            sync.dma_start(out[:16, :], out_sb[:16, :]).then_inc(out_sem, 16)

        @block.scalar
        def _(scalar):
            for c in _ring_chunks("b")[1:]:
                _chunk_dma(scalar, c, "b")
            # The scalar engine's first ACTIVATE triggers a one-time
            # ACT_TABLE_LOAD (~1.3 us); run a dummy copy mid-stream so the
            # table is hot before the tail-critical psum copies. Gated on a
            # chunk receipt so it also cannot race the gpsimd const-AP
            # memsets now that the constructor barrier is skipped.
            scalar.wait_ge(a_sem, 16)
            scalar.copy(warm_sb[0:1, B : B + 1], warm_sb[0:1, 0:1])
            # Half-1 psum->f16 copies run here, in parallel with the vector
            # engine's half-0 copies; after group 4's copy, this queue's own
            # output DMA trigger follows immediately in program order.
            for g in range(N_GROUPS):
                scalar.wait_ge(mm_sem, g + 1)
                _copy(scalar, g, 1, copy_b_sem)
            scalar.wait_ge(copy_a_sem, N_GROUPS)
            scalar.dma_start(out[16:, :], out_sb[16:, :]).then_inc(out_sem, 16)

        @block.tensor
        def _(tensor):
            # Dummy matmuls (garbage data, dead psum bank) to keep the PE
            # busy while inputs stream in, so real matmuls run at 2.4 GHz.
            for _ in range(WARMUP_MMS):
                tensor.matmul(
                    warm_ps[:], warm_sb[:, :B], warm_sb[:, B:], start=True, stop=True
                )
            t = 0
            chunk = -1
            pending_inc = 0  # groups whose psum is complete once a later MM runs
            for g, gsz in enumerate(GROUP_SIZES):
                for i in range(gsz):
                    while chunk < len(chunks) - 1 and t >= chunks[chunk + 1][0]:
                        chunk += 1
                        for sem, thr in chunk_waits[chunk]:
                            tensor.wait_ge(sem, thr)
                    mm = tensor.matmul(
                        pss[g][:],
                        data_sb[:, xoff[t] : xoff[t] + B],
                        data_sb[:, voff[t] : voff[t] + OUT_DIM],
                        start=(i == 0),
                        stop=(i == gsz - 1),
                    )
                    # Signal group g-1 complete from group g's FIRST matmul:
                    # by the time this matmul retires, the previous group's
                    # last psum writes have fully drained through the PE pipe
                    # (in-order array). Inc'ing on a group's own last matmul
                    # can fire before its drain lands -> PSUM collision when
                    # the DVE copy reads that bank.
                    if i == 0 and pending_inc:
                        mm.then_inc(mm_sem, pending_inc)
                        pending_inc = 0
                    t += 1
                pending_inc += 1
            # Final group(s): a PE drain completes only once all psum writes
            # have landed. (Signaling from a dummy matmul's retire instead —
            # to dodge the drain's ~1.15 us — measured ~2.5 us WORSE end to
            # end, consistently; the drain stays.)
            tensor.drain().then_inc(mm_sem, pending_inc)

        @block.vector
        def _(vector):
            for g in range(N_GROUPS):
                vector.wait_ge(mm_sem, g + 1)
                _copy(vector, g, 0, copy_a_sem)

        @block.gpsimd
        def _(gpsimd):
            # HAM clock phasing: the core idles at half clock; ~3.4 us of
            # continuous engine busy earns a 3.4 us full-clock boost (then a
            # forced ~6.8 us half-clock cooldown). Accumulate the busy time
            # on the otherwise-idle gpsimd, gated on chunk 0's receipt, so
            # the boost window lands on the stream tail + output chain
            # instead of being burned by PE warmups during the DMA ramp-in.
            if HAM_COLS:
                gpsimd.wait_ge(a_sem, 16)
                # Medium memsets (~0.4 us each at half clock), not one big
                # one: the boost grant only takes effect at an instruction
                # boundary, so a single 6 us memset defers its own grant to
                # its end, while too-small quanta leave dispatch gaps that
                # appear to reset the continuous-busy accumulator.
                for i in range(HAM_COLS // 256):
                    gpsimd.memset(busy_sb[:, i * 256 : (i + 1) * 256], 0)

    # After the block's end-of-kernel barrier: gpsimd alone waits for the
    # output DMA receipts (so NEFF completion implies the output is in DRAM
    # — a fresh-process first execution otherwise raced the readback), then
    # restores semaphores to zero so the NEFF can be re-executed.
    # The five non-output semaphores have no consumers after the barrier, so
    # clear them first — that work overlaps the in-flight output DMAs instead
    # of serializing after the receipt wait.
    nc.gpsimd.sem_clear(range(sem_nums[0], out_sem.num))
    nc.gpsimd.wait_ge(out_sem, 32)
    nc.gpsimd.sem_clear(range(out_sem.num, out_sem.num + 1))
    return nc


def _build_graph_tile(dt):
    dma_chunk = 5
    nc = bacc.Bacc("TRN2", target_bir_lowering=False, debug=False, num_devices=N_CORES)
    xs = nc.declare_dram_parameter("xs", [P, T_CORE * B], dt, isOutput=False)
    vs = nc.declare_dram_parameter("vs", [P, T_CORE * OUT_DIM], dt, isOutput=False)
    out = nc.declare_dram_parameter(
        "out", [B, N_GROUPS * OUT_DIM], mybir.dt.float32, isOutput=True
    )

    with tile.TileContext(nc) as tc:
        with (
            tc.tile_pool(name="data", bufs=1) as data,
            tc.tile_pool(name="psum", bufs=N_GROUPS, space="PSUM") as psum_pool,
        ):
            nchunks = -(-T_CORE // dma_chunk)
            xs_sb, vs_sb = [None] * T_CORE, [None] * T_CORE
            for c in range(nchunks):
                t0, t1 = c * dma_chunk, min((c + 1) * dma_chunk, T_CORE)
                xt = data.tile([P, (t1 - t0) * B], dt, tag=f"xs{c}")
                nc.sync.dma_start(xt[:], xs[:, t0 * B : t1 * B])
                vt = data.tile([P, (t1 - t0) * OUT_DIM], dt, tag=f"vs{c}")
                nc.sync.dma_start(vt[:], vs[:, t0 * OUT_DIM : t1 * OUT_DIM])
                for t in range(t0, t1):
                    xs_sb[t] = (xt, t - t0)
                    vs_sb[t] = (vt, t - t0)

            out_sb = data.tile([B, N_GROUPS * OUT_DIM], mybir.dt.float32, tag="out")
            t = 0
            for g, gsz in enumerate(GROUP_SIZES):
                ps = psum_pool.tile([B, OUT_DIM], mybir.dt.float32, tag="ps")
                for i in range(gsz):
                    xt, xo = xs_sb[t]
                    vt, vo = vs_sb[t]
                    nc.tensor.matmul(
                        ps[:],
                        xt[:, xo * B : (xo + 1) * B],
                        vt[:, vo * OUT_DIM : (vo + 1) * OUT_DIM],
                        start=(i == 0),
                        stop=(i == gsz - 1),
                    )
                    t += 1
                nc.vector.tensor_copy(out_sb[:, g * OUT_DIM : (g + 1) * OUT_DIM], ps[:])
            nc.sync.dma_start(out[:], out_sb[:])

    nc.compile()
    return nc


def _get_graph(dt):
    key = (MODE, dt)
    if key not in _GRAPH_CACHE:
        build = _build_graph_raw if MODE == "raw" else _build_graph_tile
        _GRAPH_CACHE[key] = build(dt)
    return _GRAPH_CACHE[key]


def _layout(masks):
    """Ring id per pixel and the ring-piece -> (core, group) slot assignment."""
    m = np.asarray(masks, dtype=np.float32).reshape(NUM_RINGS, NPIX) > 0.5
    ring = np.where(m.any(axis=0), m.argmax(axis=0), -1)

    offs = np.concatenate(([0], np.cumsum(GROUP_SIZES)))
    free = {}
    for core in range(N_CORES):
        for g, sz in enumerate(GROUP_SIZES):
            free.setdefault(sz, []).append((core, g, int(offs[g])))

    pieces = []  # (ring, core, group, core_tile_off, ring_tile_off, size)
    for r in range(NUM_RINGS):
        cnt = int((ring == r).sum())
        tiles = -(-cnt // P)
        decomp = RING_DECOMP[r]
        assert sum(decomp) == tiles, (r, cnt, tiles, decomp)
        assert cnt < tiles * P, f"ring {r} has no pad slot for the bias"
        roff = 0
        for sz in decomp:
            core, g, toff = free[sz].pop(0)
            pieces.append((r, core, g, toff, roff, sz))
            roff += sz
    return ring, pieces


def kernel(x, tokens_weights, fc_w, fc_b, masks):
    x = np.asarray(x, dtype=np.float32).reshape(B, NPIX)
    W = np.asarray(tokens_weights, dtype=np.float32).reshape(TOKEN_DIM, NPIX)
    fc_w = np.asarray(fc_w, dtype=np.float32)
    fc_b = np.asarray(fc_b, dtype=np.float32)

    # Fold the 256->192 fc into the conv weights: V[o, p] = fc_w @ W.
    V = (fc_w.astype(np.float64) @ W.astype(np.float64)).astype(np.float32)

    ring, pieces = _layout(masks)

    # Gather index per (core, tile slot, lane): pixel id, -1 pad, -2 bias.
    gidx = np.full((N_CORES, T_CORE * P), -1, dtype=np.int64)
    for r in range(NUM_RINGS):
        pix = np.nonzero(ring == r)[0]
        tiles = -(-len(pix) // P)
        arr = np.full(tiles * P, -1, dtype=np.int64)
        arr[: len(pix)] = pix
        arr[len(pix)] = -2  # bias slot (exactly one per ring)
        for rr, core, g, toff, roff, sz in pieces:
            if rr == r:
                gidx[core, toff * P : (toff + sz) * P] = arr[roff * P : (roff + sz) * P]

    sel = (gidx >= 0)[..., None]
    cl = np.clip(gidx, 0, None)
    xs_full = np.where(sel, x.T[cl], np.float32(0))  # [cores, T*P, B]
    xs_full[gidx == -2] = 1.0
    vs_full = np.where(sel, V.T[cl], np.float32(0))  # [cores, T*P, OUT_DIM]
    vs_full[gidx == -2] = fc_b

    dt_np = {
        "f16": np.float16, "bf16": ml_dtypes.bfloat16, "f32": np.float32
    }[COMPUTE_DTYPE]
    xs_dev = (
        xs_full.reshape(N_CORES, T_CORE, P, B).transpose(0, 2, 1, 3)
        .reshape(N_CORES, P, T_CORE * B).astype(dt_np)
    )
    vs_dev = (
        vs_full.reshape(N_CORES, T_CORE, P, OUT_DIM).transpose(0, 2, 1, 3)
        .reshape(N_CORES, P, T_CORE * OUT_DIM).astype(dt_np)
    )
    if MODE == "raw":
        # Fused layout: per chunk, the x columns of its tiles then the V
        # columns of its tiles — matches _sb_offsets on the device.
        data_dev = np.empty((N_CORES, P, T_CORE * TILE_COLS), dtype=dt_np)
        for t0, t1 in _chunk_bounds():
            base = t0 * TILE_COLS
            xw = (t1 - t0) * B
            data_dev[:, :, base : base + xw] = xs_dev[:, :, t0 * B : t1 * B]
            data_dev[:, :, base + xw : t1 * TILE_COLS] = vs_dev[
                :, :, t0 * OUT_DIM : t1 * OUT_DIM
            ]
        in_maps = [{"data": np.ascontiguousarray(data_dev[c])} for c in range(N_CORES)]
    else:
        in_maps = [
            {
                "xs": np.ascontiguousarray(xs_dev[c]),
                "vs": np.ascontiguousarray(vs_dev[c]),
            }
            for c in range(N_CORES)
        ]

    nc = _get_graph(mybir.dt.from_np(np.dtype(dt_np)))
    # Oracle for corruption detection: the exact per-(core, group) partials,
    # computed host-side from the same fused arrays the device consumes
    # (~1 GFLOP of f32 numpy). A degraded runtime occasionally scribbles
    # device DRAM (NaN garbage, stale buffers, or partial input clobber);
    # the returned data always comes from the device — this only decides
    # whether to re-execute.
    offs = np.concatenate(([0], np.cumsum(GROUP_SIZES)))
    prod = np.einsum(
        "ctpb,ctpo->ctbo",
        xs_full.reshape(N_CORES, T_CORE, P, B),
        vs_full.reshape(N_CORES, T_CORE, P, OUT_DIM),
        optimize=True,
    )
    exp_parts = np.add.reduceat(prod, offs[:-1], axis=1)  # [cores, groups, B, O]

    def _unpack(core_out):
        # [32, 1920] staging -> [B=64, N_GROUPS*192]: batch b is partition
        # b%32, group g in column block (2*g + b//32)*192.
        arr = core_out.reshape(OUT_P, N_GROUPS, 2, OUT_DIM)
        return arr.transpose(2, 0, 1, 3).reshape(B, N_GROUPS * OUT_DIM)

    global LAST_RESULTS
    for attempt in range(3):
        res = run_bass_kernel_spmd(
            nc, in_maps, core_ids=list(range(N_CORES)), **_RUN_KWARGS
        )
        LAST_RESULTS = res
        outs64 = [_unpack(res.results[c]["out"]) for c in range(N_CORES)]
        ok = True
        for r, core, g, toff, roff, sz in pieces:
            part = outs64[core][:, g * OUT_DIM : (g + 1) * OUT_DIM]
            part = part.astype(np.float32)
            exp = exp_parts[core, g]
            dev = np.linalg.norm(part - exp) / max(np.linalg.norm(exp), 1e-6)
            if not np.isfinite(dev) or dev > 5e-3:
                ok = False
                break
        if ok:
            break

    out = np.zeros((B, NUM_RINGS, OUT_DIM), dtype=np.float32)
    for r, core, g, toff, roff, sz in pieces:
        part = outs64[core][:, g * OUT_DIM : (g + 1) * OUT_DIM]
        out[:, r, :] += part.astype(np.float32)
    return out

